# revision 32
# baseline (speedup 1.0000x reference)
"""BiMambaBlock Trainium2 kernel (8-core SPMD), v2.

Sharding: core c -> (seq = c//2, half = c%2); seq in {b0 fwd, b0 bwd,
b1 fwd, b1 bwd}, half = 256 of the 512 d_inner channels.

v2 changes vs baseline:
 - No xproj AllReduce: every core computes xc for BOTH halves (the
   depthwise conv folds into 4 time-shifted PE matmuls with
   G_k = W_in ⊙ conv_w[:,k]), so the (48, L) projection is local.
 - Scan phase rebalanced: scans + fused scalar_tensor_tensor multiplies
   (4x DVE mode, bf16) on DVE, a tuned share of plain multiplies on
   GPSIMD, dA = exp(A_n δ) on Act, lane accumulation as pair-packed
   software-DGE accumulate DMAs.
 - Act tables batched per quarter (silu | exp+ln+exp) to avoid thrash.
 - Weights arrive as two packed SBUF blobs (2 DMAs), x as 2 padded DMAs.
 - ReduceScatter runs in bf16.
"""
import sys

for _p in ("/opt/trn_rl_repo",):
    if _p not in sys.path:
        sys.path.insert(0, _p)

import numpy as np
import ml_dtypes
from contextlib import ExitStack

import concourse.bass as bass
import concourse.tile as tile
from concourse import bacc, mybir
from concourse.bass_utils import run_bass_kernel_spmd
from concourse.masks import make_identity

BF = ml_dtypes.bfloat16
FP32 = mybir.dt.float32
BF16 = mybir.dt.bfloat16
AF = mybir.ActivationFunctionType
OP = mybir.AluOpType

# problem constants
B, L, DM = 2, 4096, 256
DI_FULL = 512
DI = 256            # own-half channels
NS = 16
RK = 16
DC = 4
H_FF = 1024
T_SLAB = 1024
NQ = 4
LQ = L // NQ        # 1024
LN_EPS = 1e-5
PJ = RK + 2 * NS    # 48
CJ = 512            # lead-in chunk
LP = L + DC - 1     # padded x length (4099)

# blob_bf column layout
GF_OFF = 0                       # gfold: pblk(2) x k(4) x db(4) x 128
GF_SZ = 2 * DC * 4 * 128         # 4096
WZ_OFF = GF_OFF + GF_SZ          # w_z: pblk(2) x 256
WZ_SZ = 2 * DI
XP_OFF = WZ_OFF + WZ_SZ          # w_xproj: db(4) x 48
XP_SZ = 4 * PJ
DT_OFF = XP_OFF + XP_SZ          # w_dt: [16 rows used] x 256
DT_SZ = DI
WO_OFF = DT_OFF + DT_SZ          # w_out: variant(2: straight/flip) x b2(2) x 256
WO_SZ = 4 * DM
FB = WO_OFF + WO_SZ
# blob_ffn column layout (loaded post-scan into a recycled scan buffer)
F1_OFF = 0                       # w_ff1: pblk(2) x 1024
F1_SZ = 2 * H_FF
F2_OFF = F1_OFF + F1_SZ          # w_ff2: m(8) x 256
F2_SZ = 8 * DM
FBF = F2_OFF + F2_SZ

# blob_f32 column layout
A_OFF = 0                        # A cols: b2(2) x 16
BD_OFF = A_OFF + 2 * NS          # b_dt: b2(2)
DP_OFF = BD_OFF + 2              # Dp: b2(2)
CB_OFF = DP_OFF + 2              # conv_b: db(4)
BF1_OFF = CB_OFF + 4             # b_ff1: 8
BF2_OFF = BF1_OFF + 8            # b_ff2 row-bcast: 256
G2_OFF = BF2_OFF + DM            # g2 row-bcast: 256
BE2_OFF = G2_OFF + DM            # beta2 row-bcast: 256
SEL_OFF = BE2_OFF + DM           # sel: 2
FF = SEL_OFF + 2

# scan state processing order (k = n+1); squares ride any act table.
# pairs of consecutive positions share one B/C broadcast DMA.
K_SEQ = [8, 16, 1, 2, 4, 3, 6, 12, 5, 10, 7, 14, 9, 11, 13, 15]
SQ_SRC = {16: 8, 2: 1, 4: 2, 6: 3, 12: 6, 10: 5, 14: 7}   # k -> source k

_NC_CACHE = {}


def _gcol(pblk, k, db):
    return GF_OFF + pblk * (DC * 4 * 128) + k * (4 * 128) + db * 128


def _inp(nc, name, shape, dtype):
    return nc.dram_tensor(name, shape, dtype, kind="ExternalInput").ap()


def _out(nc, name, shape, dtype):
    return nc.dram_tensor(name, shape, dtype, kind="ExternalOutput").ap()


def _bcast(row_ap, parts=128):
    """AP replicating a DRAM row-pair slice across `parts` partitions."""
    return bass.AP(tensor=row_ap.tensor, offset=row_ap.offset,
                   ap=[[0, parts]] + row_ap.ap)


class _Split:
    def __init__(self, frac):
        self.frac = frac
        self.acc = 0.0

    def pool(self):
        self.acc += self.frac
        if self.acc >= 1.0:
            self.acc -= 1.0
            return True
        return False


def build_program(reps=1):
    nc = bacc.Bacc("TRN2", target_bir_lowering=False, debug=False, num_devices=8)

    xT = _inp(nc, "xT", [2 * 128, LP], BF16)          # padded, flipped if bwd
    blob_bf = _inp(nc, "blob_bf", [128, FB], BF16)
    blob_ffn = _inp(nc, "blob_ffn", [128, FBF], BF16)
    blob_f32 = _inp(nc, "blob_f32", [128, FF], FP32)
    out_slab = _out(nc, "out_slab", [T_SLAB, DM], FP32)

    bc_dram = nc.dram_tensor("bc_dram", [2 * NS, L], BF16)
    rs_in = nc.dram_tensor("rs_in", [L, DM], BF16)
    rs_out = nc.dram_tensor("rs_out", [T_SLAB, DM], BF16)

    with tile.TileContext(nc) as tc, ExitStack() as ctx:
        consts = ctx.enter_context(tc.tile_pool(name="consts", bufs=1))
        persist = ctx.enter_context(tc.tile_pool(name="persist", bufs=1))
        qpool = ctx.enter_context(tc.tile_pool(name="qpool", bufs=2))
        work = ctx.enter_context(tc.tile_pool(name="work", bufs=1))
        scanp = ctx.enter_context(tc.tile_pool(name="scanp", bufs=3))
        hqp = ctx.enter_context(tc.tile_pool(name="hqp", bufs=2))
        psum = ctx.enter_context(tc.tile_pool(name="psum", bufs=8, space="PSUM"))

        wb = consts.tile([128, FB], BF16, tag="wb", name="wb")
        nc.sync.dma_start(out=wb, in_=blob_bf[:, :])
        wf = consts.tile([128, FF], FP32, tag="wf", name="wf")
        nc.sync.dma_start(out=wf, in_=blob_f32[:, :])
        eps_sb = consts.tile([128, 1], FP32, tag="eps", name="eps")
        nc.vector.memset(eps_sb, LN_EPS)
        identity_bf = consts.tile([128, 128], BF16, tag="idn", name="idn")
        make_identity(nc, identity_bf)

        for _rep in range(reps):
            xTp = [persist.tile([128, LP], BF16, tag=f"xtp{p}", name=f"xtp{p}")
                   for p in range(2)]
            for p in range(2):
                nc.sync.dma_start(out=xTp[p], in_=xT[p * 128:(p + 1) * 128, :])

            yfin = [persist.tile([128, L], BF16, tag=f"yf{b}", name=f"yf{b}")
                    for b in range(2)]
            lanes = [persist.tile([128, 4 * LQ], BF16, tag=f"lq{b}", name=f"lq{b}")
                     for b in range(2)]
            h_last = persist.tile([128, 2 * NS], FP32, tag="hl", name="hl")

            def mul_tt(out, a, b2_):
                nc.vector.tensor_tensor(out, a, b2_, OP.mult)

            def add_tt(out, a, b2_):
                nc.vector.tensor_tensor(out, a, b2_, OP.add)

            # per-quarter rotating slab state (filled by emit_leadin)
            slabs = {}

            def emit_leadin(q):
                """xc/z/proj/dt for quarter q (chunks 2q, 2q+1)."""
                qsl = slice(q * LQ, (q + 1) * LQ)
                xc_q = [qpool.tile([128, LQ], BF16, tag=f"xcq{b}", name=f"xcq{b}")
                        for b in range(2)]
                z_q = [qpool.tile([128, LQ], BF16, tag=f"zq{b}", name=f"zq{b}")
                       for b in range(2)]
                m_q = [qpool.tile([128, LQ], BF16, tag=f"mq{b}", name=f"mq{b}",
                                  bufs=1) for b in range(2)]
                bc_q = qpool.tile([PJ, LQ], BF16, tag="bcq", name="bcq", bufs=1)
                for jj in range(2):
                    j = 2 * q + jj
                    csl = slice(jj * CJ, (jj + 1) * CJ)
                    xc_ch = []
                    for db in range(4):
                        ps = psum.tile([128, CJ], FP32, tag="mm", name="mm")
                        first = True
                        for p in range(2):
                            for k in range(DC):
                                nc.tensor.matmul(
                                    ps, wb[:, _gcol(p, k, db):_gcol(p, k, db) + 128],
                                    xTp[p][:, j * CJ + k:j * CJ + k + CJ],
                                    start=first, stop=(p == 1 and k == DC - 1))
                                first = False
                        if db < 2:
                            dst = xc_q[db][:, csl]
                        else:
                            dst = work.tile([128, CJ], BF16, tag=f"xco{db}",
                                            name=f"xco{db}", bufs=2)
                        nc.scalar.activation(dst, ps, AF.Silu,
                                             bias=wf[:, CB_OFF + db:CB_OFF + db + 1])
                        xc_ch.append(dst)
                    for b2 in range(2):
                        ps = psum.tile([128, CJ], FP32, tag="mm", name="mm")
                        for p in range(2):
                            nc.tensor.matmul(
                                ps, wb[:, WZ_OFF + p * DI + b2 * 128:
                                       WZ_OFF + p * DI + b2 * 128 + 128],
                                xTp[p][:, DC - 1 + j * CJ:DC - 1 + j * CJ + CJ],
                                start=(p == 0), stop=(p == 1))
                        nc.scalar.activation(z_q[b2][:, csl], ps, AF.Silu)
                    # xproj (full 512 channels, local)
                    ps = psum.tile([128, CJ], FP32, tag="mm", name="mm")[0:PJ, :]
                    for db in range(4):
                        nc.tensor.matmul(ps, wb[:, XP_OFF + db * PJ:
                                                XP_OFF + (db + 1) * PJ],
                                         xc_ch[db], start=(db == 0), stop=(db == 3))
                    nc.scalar.copy(bc_q[:, csl], ps)
                    # dt matmul -> m staging (exp/ln batched later)
                    for b2 in range(2):
                        ps = psum.tile([128, CJ], FP32, tag="mm", name="mm")
                        nc.tensor.matmul(ps, wb[0:16, DT_OFF + b2 * 128:
                                                DT_OFF + b2 * 128 + 128],
                                         bc_q[0:RK, csl], start=True, stop=True)
                        nc.scalar.copy(m_q[b2][:, csl], ps)
                # stage B/C rows to DRAM for partition-broadcast reads
                nc.sync.dma_start(out=bc_dram[:, qsl], in_=bc_q[RK:PJ, :])
                slabs[q] = dict(xc=xc_q, z=z_q, m=m_q)

            def emit_delta(q):
                """delta = ln(1 + exp(m + b_dt)), dx = delta*xc. Emits the
                two Exp's (same act table as the scan's dA exps), then Ln."""
                s = slabs[q]
                d_q = [qpool.tile([128, LQ], BF16, tag=f"dq{b}", name=f"dq{b}")
                       for b in range(2)]
                dx_q = [qpool.tile([128, LQ], BF16, tag=f"dxq{b}", name=f"dxq{b}")
                        for b in range(2)]
                eus = []
                for b2 in range(2):
                    eu = work.tile([128, LQ], BF16, tag="eu", name="eu", bufs=1)
                    nc.scalar.activation(eu, s["m"][b2], AF.Exp,
                                         bias=wf[:, BD_OFF + b2:BD_OFF + b2 + 1])
                    eus.append(eu)
                for b2 in range(2):
                    nc.scalar.activation(d_q[b2], eus[b2], AF.Ln, bias=1.0)
                    mul_tt(dx_q[b2], d_q[b2], s["xc"][b2])
                s["d"] = d_q
                s["dx"] = dx_q

            ysel_t = {}

            def emit_wout(c):
                """ysel + rs_in rows for token chunk c (sel applied on DVE;
                flip reads yfin mirrored)."""
                csl = slice(c * LQ, (c + 1) * LQ)
                fsl = slice(L - (c + 1) * LQ, L - c * LQ)
                for b2 in range(2):
                    if b2 not in ysel_t:
                        ysel_t[b2] = persist.tile([128, L], BF16,
                                                  tag=f"xtp{b2}",
                                                  name=f"ysel{b2}")
                    ys = ysel_t[b2]
                    t = work.tile([128, LQ], BF16, tag="s1", name="ysl0",
                                  bufs=1)
                    nc.vector.tensor_scalar(t, yfin[b2][:, csl],
                                            wf[:, SEL_OFF:SEL_OFF + 1],
                                            None, OP.mult)
                    tf = work.tile([128, LQ], BF16, tag="s2", name="ysl1",
                                   bufs=1)
                    nc.vector.tensor_scalar(tf, yfin[b2][:, fsl][:, ::-1],
                                            wf[:, SEL_OFF + 1:SEL_OFF + 2],
                                            None, OP.mult)
                    add_tt(ys[:, csl], t, tf)
                for t8 in range(c * 8, (c + 1) * 8):
                    tsl = slice(t8 * 128, (t8 + 1) * 128)
                    ps = psum.tile([128, CJ], FP32, tag="mm", name="mm")[:, 0:DM]
                    for b2 in range(2):
                        nc.tensor.matmul(
                            ps, ysel_t[b2][:, tsl],
                            wb[:, WO_OFF + b2 * DM:WO_OFF + (b2 + 1) * DM],
                            start=(b2 == 0), stop=(b2 == 1))
                    yp = work.tile([128, DM], BF16, tag="ypart", name="ypart",
                                   bufs=3)
                    nc.scalar.copy(yp, ps)
                    nc.sync.dma_start(out=rs_in[tsl, :], in_=yp)

            emit_leadin(0)
            emit_delta(0)
            wout_pending = [0, 1, 2, 3]

            for q in range(NQ):
                qsl = slice(q * LQ, (q + 1) * LQ)
                s = slabs.pop(q)
                xc_q, z_q, d_q, dx_q = s["xc"], s["z"], s["d"], s["dx"]
                # ---- selective scan over 16 states (B/C fetched in pairs,
                # processed in K_SEQ order; even decays from Act Square) ----
                hq_cur = [None, None]
                hc_split = _Split(0.81)    # share of hc multiplies on GPSIMD
                dA_prev = [{}, {}]         # k -> dA tile (square sources)
                for pos in range(NS):
                    n = K_SEQ[pos] - 1
                    if pos % 2 == 0:
                        bc = scanp.tile([128, 4 * LQ], BF16, tag="bc", name="bc",
                                        bufs=3)
                        # rows 4p..4p+3 = pair-grouped (host-permuted to K_SEQ)
                        row = bc_dram[2 * pos:2 * pos + 1, qsl]
                        nc.sync.dma_start(
                            out=bc,
                            in_=bass.AP(tensor=row.tensor, offset=row.offset,
                                        ap=[[0, 128], [L, 4]] + row.ap[1:]))
                    bco = (pos % 2) * LQ
                    for b2 in range(2):
                        hcol = NS * b2 + n
                        dA = scanp.tile([128, LQ], BF16, tag=f"dA{b2}",
                                        name=f"dA{b2}", bufs=4)
                        k = n + 1
                        if k in SQ_SRC:
                            nc.scalar.activation(dA, dA_prev[b2][SQ_SRC[k]],
                                                 AF.Square)
                        else:
                            nc.scalar.activation(
                                dA, d_q[b2], AF.Exp,
                                scale=wf[:, A_OFF + b2 * NS + n:
                                         A_OFF + b2 * NS + n + 1])
                        dA_prev[b2][k] = dA
                        dBx = scanp.tile([128, LQ], BF16, tag=f"dBx{b2}",
                                         name=f"dBx{b2}", bufs=3)
                        nc.vector.tensor_tensor(dBx, dx_q[b2], bc[:, bco:bco + LQ],
                                                OP.mult)
                        h = scanp.tile([128, LQ], BF16, tag=f"h{b2}", name=f"h{b2}")
                        init = 0.0 if q == 0 else h_last[:, hcol:hcol + 1]
                        nc.vector.tensor_tensor_scan(h, dA, dBx, init,
                                                     OP.mult, OP.add)
                        if q < NQ - 1:
                            nc.gpsimd.tensor_copy(h_last[:, hcol:hcol + 1],
                                                  h[:, LQ - 1:LQ])
                        hc_eng = (nc.gpsimd if hc_split.pool() else nc.vector)
                        if pos < 4:
                            hc_eng.tensor_tensor(
                                lanes[b2][:, pos * LQ:(pos + 1) * LQ],
                                h, bc[:, 2 * LQ + bco:2 * LQ + bco + LQ], OP.mult)
                        else:
                            slot = (pos - 4) % 2
                            if slot == 0:
                                hq_cur[b2] = hqp.tile([128, 2 * LQ], BF16,
                                                      tag=f"hq{b2}",
                                                      name=f"hq{b2}")
                            hc_eng.tensor_tensor(
                                hq_cur[b2][:, slot * LQ:(slot + 1) * LQ],
                                h, bc[:, 2 * LQ + bco:2 * LQ + bco + LQ], OP.mult)
                            if slot == 1:
                                base = (pos - 5) % 4
                                nc.gpsimd.dma_start(
                                    out=lanes[b2][:, base * LQ:(base + 2) * LQ],
                                    in_=hq_cur[b2], accum_op=OP.add)
                # next quarter's lead-in + exp/ln emitted after the scan so the
                # act-queue batches by table (exps | silus | exp+ln)
                if q + 1 < NQ:
                    emit_leadin(q + 1)
                    emit_delta(q + 1)
                # ---- finish quarter: y = (xc*Dp + sum lanes) * silu(z) ----
                for b2 in range(2):
                    s1 = work.tile([128, LQ], BF16, tag="s1", name="s1", bufs=1)
                    add_tt(s1, lanes[b2][:, 0:LQ], lanes[b2][:, LQ:2 * LQ])
                    s2 = work.tile([128, LQ], BF16, tag="s2", name="s2", bufs=1)
                    add_tt(s2, lanes[b2][:, 2 * LQ:3 * LQ],
                           lanes[b2][:, 3 * LQ:4 * LQ])
                    xcdp = work.tile([128, LQ], BF16, tag="t1", name="xcdp", bufs=1)
                    nc.vector.tensor_scalar(xcdp, xc_q[b2],
                                            wf[:, DP_OFF + b2:DP_OFF + b2 + 1],
                                            None, OP.mult)
                    t1 = work.tile([128, LQ], BF16, tag="t1b", name="t1b", bufs=1)
                    add_tt(t1, xcdp, s1)
                    t2 = work.tile([128, LQ], BF16, tag="t2", name="t2", bufs=1)
                    add_tt(t2, t1, s2)
                    mul_tt(yfin[b2][:, qsl], t2, z_q[b2])
                if q == 2:
                    # middle output chunks need only yfin quarters 1,2
                    for c in (1, 2):
                        emit_wout(c)
                        wout_pending.remove(c)

            # ---- W_out (sel folded into straight/flipped weights on host) ----
            for c in wout_pending:
                emit_wout(c)
            wffn = scanp.tile([128, FBF], BF16, tag="bc", name="wffn")
            nc.sync.dma_start(out=wffn, in_=blob_ffn[:, :])
            nc.gpsimd.collective_compute(
                "ReduceScatter", OP.add,
                replica_groups=[[0, 1, 2, 3], [4, 5, 6, 7]],
                ins=[rs_in[:, :]], outs=[rs_out[:, :]])

            # ---- FFN + layernorm on the 1024-token slab ----
            ysb_bf = persist.tile([128, 8 * DM], BF16, tag="xtp1", name="ysbb")
            ro = rs_out[0:128, :]
            nc.sync.dma_start(
                out=ysb_bf,
                in_=bass.AP(tensor=ro.tensor, offset=ro.offset,
                            ap=[ro.ap[0], [128 * DM, 8]] + ro.ap[1:]))
            ysb_all = persist.tile([128, 8 * DM], FP32, tag="xtp0", name="ysba")
            ysb = [ysb_bf[:, t8 * DM:(t8 + 1) * DM] for t8 in range(8)]
            ysbf = [ysb_all[:, t8 * DM:(t8 + 1) * DM] for t8 in range(8)]
            yT_bf = [hqp.tile([128, T_SLAB * 2], BF16, tag=f"hq{k}",
                              name=f"yT{k}")[:, 0:T_SLAB] for k in range(2)]
            nc.vector.tensor_copy(ysb_all, ysb_bf)
            for t8 in range(8):
                for k in range(2):
                    pst = psum.tile([128, CJ * 2], BF16, tag="mm",
                                    name="mmtr")[:, 0:128]
                    nc.tensor.transpose(pst, ysb[t8][:, k * 128:(k + 1) * 128],
                                        identity_bf)
                    nc.scalar.copy(yT_bf[k][:, t8 * 128:(t8 + 1) * 128], pst)
            # gelu tiles alias the yfin slabs (dead after ysel)
            g_sb = [yfin[m // 4][:, (m % 4) * T_SLAB:(m % 4 + 1) * T_SLAB]
                    for m in range(8)]
            for m in range(8):
                for jj in range(2):
                    ps = psum.tile([128, CJ], FP32, tag="mm", name="mmf")
                    for k in range(2):
                        nc.tensor.matmul(
                            ps,
                            wffn[:, F1_OFF + k * H_FF + m * 128:
                                 F1_OFF + k * H_FF + (m + 1) * 128],
                            yT_bf[k][:, jj * 512:(jj + 1) * 512],
                            start=(k == 0), stop=(k == 1))
                    nc.scalar.activation(g_sb[m][:, jj * 512:(jj + 1) * 512], ps,
                                         AF.Gelu,
                                         bias=wf[:, BF1_OFF + m:BF1_OFF + m + 1])
            for t8 in range(8):
                tsl = slice(t8 * 128, (t8 + 1) * 128)
                ps = psum.tile([128, CJ], FP32, tag="mm", name="mm")[:, 0:DM]
                for m in range(8):
                    nc.tensor.matmul(ps, g_sb[m][:, tsl],
                                     wffn[:, F2_OFF + m * DM:F2_OFF + (m + 1) * DM],
                                     start=(m == 0), stop=(m == 7))
                ff = work.tile([128, DM], FP32, tag="ff", name="ff", bufs=2)
                nc.vector.tensor_tensor(ff, ps, wf[:, BF2_OFF:BF2_OFF + DM], OP.add)
                stats = work.tile([128, 6], FP32, tag="stats", name="stats", bufs=2)
                nc.vector.bn_stats(stats, ff)
                mv = work.tile([128, 2], FP32, tag="mv", name="mv", bufs=2)
                nc.vector.bn_aggr(mv, stats)
                sq = work.tile([128, 1], FP32, tag="sq", name="sq", bufs=2)
                nc.scalar.activation(sq, mv[:, 1:2], AF.Sqrt, bias=eps_sb[:, 0:1])
                rstd = work.tile([128, 1], FP32, tag="rstd", name="rstd", bufs=2)
                nc.vector.reciprocal(rstd, sq)
                norm = work.tile([128, DM], FP32, tag="norm", name="norm", bufs=2)
                nc.vector.tensor_scalar(norm, ff, mv[:, 0:1], rstd,
                                        OP.subtract, OP.mult)
                nc.vector.tensor_tensor(norm, norm, wf[:, G2_OFF:G2_OFF + DM],
                                        OP.mult)
                nc.vector.tensor_tensor(norm, norm, wf[:, BE2_OFF:BE2_OFF + DM],
                                        OP.add)
                nc.vector.tensor_tensor(norm, norm, ysbf[t8], OP.add)
                nc.sync.dma_start(out=out_slab[tsl, :], in_=norm)

    nc.compile()
    return nc


def _prep_inputs(inputs):
    x = np.asarray(inputs["x"], np.float32)
    W_in = np.asarray(inputs["W_in"], np.float32)
    conv_w = np.asarray(inputs["conv_w"], np.float32)
    conv_b = np.asarray(inputs["conv_b"], np.float32)
    W_xproj = np.asarray(inputs["W_xproj"], np.float32)
    W_dt = np.asarray(inputs["W_dt"], np.float32)
    b_dt = np.asarray(inputs["b_dt"], np.float32)
    A_log = np.asarray(inputs["A_log"], np.float32)
    Dp = np.asarray(inputs["Dp"], np.float32)
    W_out = np.asarray(inputs["W_out"], np.float32)
    W_ff1 = np.asarray(inputs["W_ff1"], np.float32)
    b_ff1 = np.asarray(inputs["b_ff1"], np.float32)
    W_ff2 = np.asarray(inputs["W_ff2"], np.float32)
    b_ff2 = np.asarray(inputs["b_ff2"], np.float32)
    g2 = np.asarray(inputs["g2"], np.float32)
    beta2 = np.asarray(inputs["beta2"], np.float32)

    A = -np.exp(A_log)  # (512, 16)

    # gfold: G_k[m, d] = W_in[m, d] * conv_w[d, k]   (xs columns, both halves)
    W_xs = W_in[:, :DI_FULL]
    gf = np.zeros((128, GF_SZ), np.float32)
    for p in range(2):
        for k in range(DC):
            for db in range(4):
                off = p * (DC * 4 * 128) + k * (4 * 128) + db * 128
                gf[:, off:off + 128] = (
                    W_xs[p * 128:(p + 1) * 128, db * 128:(db + 1) * 128]
                    * conv_w[db * 128:(db + 1) * 128, k][None, :])

    in_maps = []
    for c in range(8):
        seq, half = c // 2, c % 2
        bb, bwd = seq // 2, seq % 2
        xs_np = x[bb]
        if bwd:
            xs_np = xs_np[::-1]
        xpad = np.zeros((DM, LP), np.float32)
        xpad[:, DC - 1:] = xs_np.T
        hsl = slice(half * DI, (half + 1) * DI)

        # d-block order: own half's two blocks first (the kernel scans
        # blocks 0,1 with the per-half parameters)
        dbo = [0, 1, 2, 3] if half == 0 else [2, 3, 0, 1]
        blob = np.zeros((128, FB), np.float32)
        for p in range(2):
            for k in range(DC):
                for dbi, db in enumerate(dbo):
                    off_src = p * (DC * 4 * 128) + k * (4 * 128) + db * 128
                    off_dst = p * (DC * 4 * 128) + k * (4 * 128) + dbi * 128
                    blob[:, GF_OFF + off_dst:GF_OFF + off_dst + 128] = \
                        gf[:, off_src:off_src + 128]
        for p in range(2):
            blob[:, WZ_OFF + p * DI:WZ_OFF + (p + 1) * DI] = \
                W_in[p * 128:(p + 1) * 128,
                     DI_FULL + half * DI:DI_FULL + (half + 1) * DI]
        # B/C columns pair-grouped to match the K_SEQ processing order
        perm = list(range(RK))
        for p in range(NS // 2):
            n0, n1 = K_SEQ[2 * p] - 1, K_SEQ[2 * p + 1] - 1
            perm += [RK + n0, RK + n1, RK + NS + n0, RK + NS + n1]
        W_xp_perm = W_xproj[:, perm]
        for dbi, db in enumerate(dbo):
            blob[:, XP_OFF + dbi * PJ:XP_OFF + (dbi + 1) * PJ] = \
                W_xp_perm[db * 128:(db + 1) * 128, :]
        blob[0:RK, DT_OFF:DT_OFF + DI] = W_dt[:, hsl]
        for b2 in range(2):
            blob[:, WO_OFF + b2 * DM:WO_OFF + (b2 + 1) * DM] = \
                W_out[half * DI + b2 * 128:half * DI + (b2 + 1) * 128, :]
        fb = np.zeros((128, FBF), np.float32)
        for p in range(2):
            fb[:, F1_OFF + p * H_FF:F1_OFF + (p + 1) * H_FF] = \
                W_ff1[p * 128:(p + 1) * 128, :]
        for m in range(8):
            fb[:, F2_OFF + m * DM:F2_OFF + (m + 1) * DM] = \
                W_ff2[m * 128:(m + 1) * 128, :]

        fblob = np.zeros((128, FF), np.float32)
        for b2 in range(2):
            fblob[:, A_OFF + b2 * NS:A_OFF + (b2 + 1) * NS] = \
                A[half * DI + b2 * 128:half * DI + (b2 + 1) * 128, :]
            fblob[:, BD_OFF + b2] = b_dt[half * DI + b2 * 128:
                                         half * DI + (b2 + 1) * 128]
            fblob[:, DP_OFF + b2] = Dp[half * DI + b2 * 128:
                                       half * DI + (b2 + 1) * 128]
        for dbi, db in enumerate(dbo):
            fblob[:, CB_OFF + dbi] = conv_b[db * 128:(db + 1) * 128]
        fblob[:, BF1_OFF:BF1_OFF + 8] = b_ff1.reshape(8, 128).T
        fblob[:, BF2_OFF:BF2_OFF + DM] = b_ff2[None, :]
        fblob[:, G2_OFF:G2_OFF + DM] = g2[None, :]
        fblob[:, BE2_OFF:BE2_OFF + DM] = beta2[None, :]
        fblob[:, SEL_OFF] = 1.0 - bwd
        fblob[:, SEL_OFF + 1] = float(bwd)

        in_maps.append(dict(
            xT=xpad.astype(BF),
            blob_bf=blob.astype(BF),
            blob_ffn=fb.astype(BF),
            blob_f32=fblob,
        ))
    return in_maps


def kernel(**inputs) -> np.ndarray:
    if "nc" not in _NC_CACHE:
        _NC_CACHE["nc"] = build_program()
    nc = _NC_CACHE["nc"]
    in_maps = _prep_inputs(inputs)
    res = run_bass_kernel_spmd(nc, in_maps, core_ids=list(range(8)))
    out = np.empty((B, L, DM), np.float32)
    for c in range(8):
        bb = c // 4
        r = c % 4
        out[bb, r * T_SLAB:(r + 1) * T_SLAB, :] = res.results[c]["out_slab"]
    return out


# revision 33
# speedup vs baseline: 1.9952x; 1.9952x over previous
"""BiMambaBlock Trainium2 kernel (8-core SPMD), v2.

Sharding: core c -> (seq = c//2, half = c%2); seq in {b0 fwd, b0 bwd,
b1 fwd, b1 bwd}, half = 256 of the 512 d_inner channels.

v2 changes vs baseline:
 - No xproj AllReduce: every core computes xc for BOTH halves (the
   depthwise conv folds into 4 time-shifted PE matmuls with
   G_k = W_in ⊙ conv_w[:,k]), so the (48, L) projection is local.
 - Scan phase rebalanced: scans + fused scalar_tensor_tensor multiplies
   (4x DVE mode, bf16) on DVE, a tuned share of plain multiplies on
   GPSIMD, dA = exp(A_n δ) on Act, lane accumulation as pair-packed
   software-DGE accumulate DMAs.
 - Act tables batched per quarter (silu | exp+ln+exp) to avoid thrash.
 - Weights arrive as two packed SBUF blobs (2 DMAs), x as 2 padded DMAs.
 - ReduceScatter runs in bf16.
"""
import sys

for _p in ("/opt/trn_rl_repo",):
    if _p not in sys.path:
        sys.path.insert(0, _p)

import numpy as np
import ml_dtypes
from contextlib import ExitStack

import concourse.bass as bass
import concourse.tile as tile
from concourse import bacc, mybir
from concourse.bass_utils import run_bass_kernel_spmd
from concourse.masks import make_identity

BF = ml_dtypes.bfloat16
FP32 = mybir.dt.float32
BF16 = mybir.dt.bfloat16
AF = mybir.ActivationFunctionType
OP = mybir.AluOpType

# problem constants
B, L, DM = 2, 4096, 256
DI_FULL = 512
DI = 256            # own-half channels
NS = 16
RK = 16
DC = 4
H_FF = 1024
T_SLAB = 1024
NQ = 4
LQ = L // NQ        # 1024
LN_EPS = 1e-5
PJ = RK + 2 * NS    # 48
CJ = 512            # lead-in chunk
LP = L + DC - 1     # padded x length (4099)

# blob_bf column layout
GF_OFF = 0                       # gfold: pblk(2) x k(4) x db(4) x 128
GF_SZ = 2 * DC * 4 * 128         # 4096
WZ_OFF = GF_OFF + GF_SZ          # w_z: pblk(2) x 256
WZ_SZ = 2 * DI
XP_OFF = WZ_OFF + WZ_SZ          # w_xproj: db(4) x 48
XP_SZ = 4 * PJ
DT_OFF = XP_OFF + XP_SZ          # w_dt: [16 rows used] x 256
DT_SZ = DI
WO_OFF = DT_OFF + DT_SZ          # w_out: variant(2: straight/flip) x b2(2) x 256
WO_SZ = 4 * DM
FB = WO_OFF + WO_SZ
# blob_ffn column layout (loaded post-scan into a recycled scan buffer)
F1_OFF = 0                       # w_ff1: pblk(2) x 1024
F1_SZ = 2 * H_FF
F2_OFF = F1_OFF + F1_SZ          # w_ff2: m(8) x 256
F2_SZ = 8 * DM
FBF = F2_OFF + F2_SZ

# blob_f32 column layout
A_OFF = 0                        # A cols: b2(2) x 16
BD_OFF = A_OFF + 2 * NS          # b_dt: b2(2)
DP_OFF = BD_OFF + 2              # Dp: b2(2)
CB_OFF = DP_OFF + 2              # conv_b: db(4)
BF1_OFF = CB_OFF + 4             # b_ff1: 8
BF2_OFF = BF1_OFF + 8            # b_ff2 row-bcast: 256
G2_OFF = BF2_OFF + DM            # g2 row-bcast: 256
BE2_OFF = G2_OFF + DM            # beta2 row-bcast: 256
SEL_OFF = BE2_OFF + DM           # sel: 2
FF = SEL_OFF + 2

# scan state processing order (k = n+1); squares ride any act table.
# pairs of consecutive positions share one B/C broadcast DMA.
K_SEQ = [8, 16, 1, 2, 4, 3, 6, 12, 5, 10, 7, 14, 9, 11, 13, 15]
SQ_SRC = {16: 8, 2: 1, 4: 2, 6: 3, 12: 6, 10: 5, 14: 7}   # k -> source k

_NC_CACHE = {}


def _gcol(pblk, k, db):
    return GF_OFF + pblk * (DC * 4 * 128) + k * (4 * 128) + db * 128


def _inp(nc, name, shape, dtype):
    return nc.dram_tensor(name, shape, dtype, kind="ExternalInput").ap()


def _out(nc, name, shape, dtype):
    return nc.dram_tensor(name, shape, dtype, kind="ExternalOutput").ap()


def _bcast(row_ap, parts=128):
    """AP replicating a DRAM row-pair slice across `parts` partitions."""
    return bass.AP(tensor=row_ap.tensor, offset=row_ap.offset,
                   ap=[[0, parts]] + row_ap.ap)


class _Split:
    def __init__(self, frac):
        self.frac = frac
        self.acc = 0.0

    def pool(self):
        self.acc += self.frac
        if self.acc >= 1.0:
            self.acc -= 1.0
            return True
        return False


def build_program(reps=1):
    nc = bacc.Bacc("TRN2", target_bir_lowering=False, debug=False, num_devices=8)

    xT = _inp(nc, "xT", [2 * 128, LP], BF16)          # padded, flipped if bwd
    blob_bf = _inp(nc, "blob_bf", [128, FB], BF16)
    blob_ffn = _inp(nc, "blob_ffn", [128, FBF], BF16)
    blob_f32 = _inp(nc, "blob_f32", [128, FF], FP32)
    out_slab = _out(nc, "out_slab", [T_SLAB, DM], FP32)

    bc_dram = nc.dram_tensor("bc_dram", [2 * NS, L], BF16)
    rs_in = nc.dram_tensor("rs_in", [L, DM], FP32)
    rs_out = nc.dram_tensor("rs_out", [T_SLAB, DM], FP32)

    with tile.TileContext(nc) as tc, ExitStack() as ctx:
        consts = ctx.enter_context(tc.tile_pool(name="consts", bufs=1))
        persist = ctx.enter_context(tc.tile_pool(name="persist", bufs=1))
        qpool = ctx.enter_context(tc.tile_pool(name="qpool", bufs=2))
        work = ctx.enter_context(tc.tile_pool(name="work", bufs=1))
        scanp = ctx.enter_context(tc.tile_pool(name="scanp", bufs=3))
        hqp = ctx.enter_context(tc.tile_pool(name="hqp", bufs=2))
        psum = ctx.enter_context(tc.tile_pool(name="psum", bufs=8, space="PSUM"))

        wb = consts.tile([128, FB], BF16, tag="wb", name="wb")
        nc.sync.dma_start(out=wb, in_=blob_bf[:, :])
        wf = consts.tile([128, FF], FP32, tag="wf", name="wf")
        nc.sync.dma_start(out=wf, in_=blob_f32[:, :])
        eps_sb = consts.tile([128, 1], FP32, tag="eps", name="eps")
        nc.vector.memset(eps_sb, LN_EPS)
        identity_bf = consts.tile([128, 128], BF16, tag="idn", name="idn")
        make_identity(nc, identity_bf)

        for _rep in range(reps):
            xTp = [persist.tile([128, LP], BF16, tag=f"xtp{p}", name=f"xtp{p}")
                   for p in range(2)]
            for p in range(2):
                nc.sync.dma_start(out=xTp[p], in_=xT[p * 128:(p + 1) * 128, :])

            yfin = [persist.tile([128, L], BF16, tag=f"yf{b}", name=f"yf{b}")
                    for b in range(2)]
            lanes = [persist.tile([128, 4 * LQ], BF16, tag=f"lq{b}", name=f"lq{b}")
                     for b in range(2)]
            h_last = persist.tile([128, 2 * NS], FP32, tag="hl", name="hl")

            def mul_tt(out, a, b2_):
                nc.vector.tensor_tensor(out, a, b2_, OP.mult)

            def add_tt(out, a, b2_):
                nc.vector.tensor_tensor(out, a, b2_, OP.add)

            # per-quarter rotating slab state (filled by emit_leadin)
            slabs = {}

            def emit_leadin(q):
                """xc/z/proj/dt for quarter q (chunks 2q, 2q+1)."""
                qsl = slice(q * LQ, (q + 1) * LQ)
                xc_q = [qpool.tile([128, LQ], BF16, tag=f"xcq{b}", name=f"xcq{b}")
                        for b in range(2)]
                z_q = [qpool.tile([128, LQ], BF16, tag=f"zq{b}", name=f"zq{b}")
                       for b in range(2)]
                m_q = [qpool.tile([128, LQ], BF16, tag=f"mq{b}", name=f"mq{b}",
                                  bufs=1) for b in range(2)]
                bc_q = qpool.tile([PJ, LQ], BF16, tag="bcq", name="bcq", bufs=1)
                for jj in range(2):
                    j = 2 * q + jj
                    csl = slice(jj * CJ, (jj + 1) * CJ)
                    xc_ch = []
                    for db in range(4):
                        ps = psum.tile([128, CJ], FP32, tag="mm", name="mm")
                        first = True
                        for p in range(2):
                            for k in range(DC):
                                nc.tensor.matmul(
                                    ps, wb[:, _gcol(p, k, db):_gcol(p, k, db) + 128],
                                    xTp[p][:, j * CJ + k:j * CJ + k + CJ],
                                    start=first, stop=(p == 1 and k == DC - 1))
                                first = False
                        if db < 2:
                            dst = xc_q[db][:, csl]
                        else:
                            dst = work.tile([128, CJ], BF16, tag=f"xco{db}",
                                            name=f"xco{db}", bufs=2)
                        nc.scalar.activation(dst, ps, AF.Silu,
                                             bias=wf[:, CB_OFF + db:CB_OFF + db + 1])
                        xc_ch.append(dst)
                    for b2 in range(2):
                        ps = psum.tile([128, CJ], FP32, tag="mm", name="mm")
                        for p in range(2):
                            nc.tensor.matmul(
                                ps, wb[:, WZ_OFF + p * DI + b2 * 128:
                                       WZ_OFF + p * DI + b2 * 128 + 128],
                                xTp[p][:, DC - 1 + j * CJ:DC - 1 + j * CJ + CJ],
                                start=(p == 0), stop=(p == 1))
                        nc.scalar.activation(z_q[b2][:, csl], ps, AF.Silu)
                    # xproj (full 512 channels, local)
                    ps = psum.tile([128, CJ], FP32, tag="mm", name="mm")[0:PJ, :]
                    for db in range(4):
                        nc.tensor.matmul(ps, wb[:, XP_OFF + db * PJ:
                                                XP_OFF + (db + 1) * PJ],
                                         xc_ch[db], start=(db == 0), stop=(db == 3))
                    nc.scalar.copy(bc_q[:, csl], ps)
                    # dt matmul -> m staging (exp/ln batched later)
                    for b2 in range(2):
                        ps = psum.tile([128, CJ], FP32, tag="mm", name="mm")
                        nc.tensor.matmul(ps, wb[0:16, DT_OFF + b2 * 128:
                                                DT_OFF + b2 * 128 + 128],
                                         bc_q[0:RK, csl], start=True, stop=True)
                        nc.scalar.copy(m_q[b2][:, csl], ps)
                # stage B/C rows to DRAM for partition-broadcast reads
                nc.sync.dma_start(out=bc_dram[:, qsl], in_=bc_q[RK:PJ, :])
                slabs[q] = dict(xc=xc_q, z=z_q, m=m_q)

            def emit_delta(q):
                """delta = ln(1 + exp(m + b_dt)), dx = delta*xc. Emits the
                two Exp's (same act table as the scan's dA exps), then Ln."""
                s = slabs[q]
                d_q = [qpool.tile([128, LQ], BF16, tag=f"dq{b}", name=f"dq{b}")
                       for b in range(2)]
                dx_q = [qpool.tile([128, LQ], BF16, tag=f"dxq{b}", name=f"dxq{b}")
                        for b in range(2)]
                eus = []
                for b2 in range(2):
                    eu = work.tile([128, LQ], BF16, tag="eu", name="eu", bufs=1)
                    nc.scalar.activation(eu, s["m"][b2], AF.Exp,
                                         bias=wf[:, BD_OFF + b2:BD_OFF + b2 + 1])
                    eus.append(eu)
                for b2 in range(2):
                    nc.scalar.activation(d_q[b2], eus[b2], AF.Ln, bias=1.0)
                    mul_tt(dx_q[b2], d_q[b2], s["xc"][b2])
                s["d"] = d_q
                s["dx"] = dx_q

            ysel_t = {}

            def emit_wout(c):
                """ysel + rs_in rows for token chunk c (sel applied on DVE;
                flip reads yfin mirrored)."""
                csl = slice(c * LQ, (c + 1) * LQ)
                fsl = slice(L - (c + 1) * LQ, L - c * LQ)
                for b2 in range(2):
                    if b2 not in ysel_t:
                        ysel_t[b2] = persist.tile([128, L], BF16,
                                                  tag=f"xtp{b2}",
                                                  name=f"ysel{b2}")
                    ys = ysel_t[b2]
                    t = work.tile([128, LQ], BF16, tag="s1", name="ysl0",
                                  bufs=1)
                    nc.vector.tensor_scalar(t, yfin[b2][:, csl],
                                            wf[:, SEL_OFF:SEL_OFF + 1],
                                            None, OP.mult)
                    tf = work.tile([128, LQ], BF16, tag="s2", name="ysl1",
                                   bufs=1)
                    nc.vector.tensor_scalar(tf, yfin[b2][:, fsl][:, ::-1],
                                            wf[:, SEL_OFF + 1:SEL_OFF + 2],
                                            None, OP.mult)
                    add_tt(ys[:, csl], t, tf)
                for t8 in range(c * 8, (c + 1) * 8):
                    tsl = slice(t8 * 128, (t8 + 1) * 128)
                    ps = psum.tile([128, CJ], FP32, tag="mm", name="mm")[:, 0:DM]
                    for b2 in range(2):
                        nc.tensor.matmul(
                            ps, ysel_t[b2][:, tsl],
                            wb[:, WO_OFF + b2 * DM:WO_OFF + (b2 + 1) * DM],
                            start=(b2 == 0), stop=(b2 == 1))
                    yp = work.tile([128, DM], FP32, tag="ypart", name="ypart",
                                   bufs=3)
                    nc.scalar.copy(yp, ps)
                    nc.sync.dma_start(out=rs_in[tsl, :], in_=yp)

            emit_leadin(0)
            emit_delta(0)
            wout_pending = [0, 1, 2, 3]

            for q in range(NQ):
                qsl = slice(q * LQ, (q + 1) * LQ)
                s = slabs.pop(q)
                xc_q, z_q, d_q, dx_q = s["xc"], s["z"], s["d"], s["dx"]
                # ---- selective scan over 16 states (B/C fetched in pairs,
                # processed in K_SEQ order; even decays from Act Square) ----
                hq_cur = [None, None]
                hc_split = _Split(0.81)    # share of hc multiplies on GPSIMD
                dA_prev = [{}, {}]         # k -> dA tile (square sources)
                for pos in range(NS):
                    n = K_SEQ[pos] - 1
                    if pos % 2 == 0:
                        bc = scanp.tile([128, 4 * LQ], BF16, tag="bc", name="bc",
                                        bufs=3)
                        # rows 4p..4p+3 = pair-grouped (host-permuted to K_SEQ)
                        row = bc_dram[2 * pos:2 * pos + 1, qsl]
                        nc.sync.dma_start(
                            out=bc,
                            in_=bass.AP(tensor=row.tensor, offset=row.offset,
                                        ap=[[0, 128], [L, 4]] + row.ap[1:]))
                    bco = (pos % 2) * LQ
                    for b2 in range(2):
                        hcol = NS * b2 + n
                        dA = scanp.tile([128, LQ], BF16, tag=f"dA{b2}",
                                        name=f"dA{b2}", bufs=4)
                        k = n + 1
                        if k in SQ_SRC:
                            nc.scalar.activation(dA, dA_prev[b2][SQ_SRC[k]],
                                                 AF.Square)
                        else:
                            nc.scalar.activation(
                                dA, d_q[b2], AF.Exp,
                                scale=wf[:, A_OFF + b2 * NS + n:
                                         A_OFF + b2 * NS + n + 1])
                        dA_prev[b2][k] = dA
                        dBx = scanp.tile([128, LQ], BF16, tag=f"dBx{b2}",
                                         name=f"dBx{b2}", bufs=3)
                        nc.vector.tensor_tensor(dBx, dx_q[b2], bc[:, bco:bco + LQ],
                                                OP.mult)
                        h = scanp.tile([128, LQ], BF16, tag=f"h{b2}", name=f"h{b2}")
                        init = 0.0 if q == 0 else h_last[:, hcol:hcol + 1]
                        nc.vector.tensor_tensor_scan(h, dA, dBx, init,
                                                     OP.mult, OP.add)
                        if q < NQ - 1:
                            nc.vector.tensor_copy(h_last[:, hcol:hcol + 1],
                                                  h[:, LQ - 1:LQ])
                        hc_eng = (nc.gpsimd if hc_split.pool() else nc.vector)
                        if pos < 4:
                            hc_eng.tensor_tensor(
                                lanes[b2][:, pos * LQ:(pos + 1) * LQ],
                                h, bc[:, 2 * LQ + bco:2 * LQ + bco + LQ], OP.mult)
                        else:
                            slot = (pos - 4) % 2
                            if slot == 0:
                                hq_cur[b2] = hqp.tile([128, 2 * LQ], BF16,
                                                      tag=f"hq{b2}",
                                                      name=f"hq{b2}")
                            hc_eng.tensor_tensor(
                                hq_cur[b2][:, slot * LQ:(slot + 1) * LQ],
                                h, bc[:, 2 * LQ + bco:2 * LQ + bco + LQ], OP.mult)
                            if slot == 1:
                                base = (pos - 5) % 4
                                nc.gpsimd.dma_start(
                                    out=lanes[b2][:, base * LQ:(base + 2) * LQ],
                                    in_=hq_cur[b2], accum_op=OP.add)
                # next quarter's lead-in + exp/ln emitted after the scan so the
                # act-queue batches by table (exps | silus | exp+ln)
                if q + 1 < NQ:
                    emit_leadin(q + 1)
                    emit_delta(q + 1)
                # ---- finish quarter: y = (xc*Dp + sum lanes) * silu(z) ----
                for b2 in range(2):
                    s1 = work.tile([128, LQ], BF16, tag="s1", name="s1", bufs=1)
                    add_tt(s1, lanes[b2][:, 0:LQ], lanes[b2][:, LQ:2 * LQ])
                    s2 = work.tile([128, LQ], BF16, tag="s2", name="s2", bufs=1)
                    add_tt(s2, lanes[b2][:, 2 * LQ:3 * LQ],
                           lanes[b2][:, 3 * LQ:4 * LQ])
                    xcdp = work.tile([128, LQ], BF16, tag="t1", name="xcdp", bufs=1)
                    nc.vector.tensor_scalar(xcdp, xc_q[b2],
                                            wf[:, DP_OFF + b2:DP_OFF + b2 + 1],
                                            None, OP.mult)
                    t1 = work.tile([128, LQ], BF16, tag="t1b", name="t1b", bufs=1)
                    add_tt(t1, xcdp, s1)
                    t2 = work.tile([128, LQ], BF16, tag="t2", name="t2", bufs=1)
                    add_tt(t2, t1, s2)
                    mul_tt(yfin[b2][:, qsl], t2, z_q[b2])
                if q == 2:
                    # middle output chunks need only yfin quarters 1,2
                    for c in (1, 2):
                        emit_wout(c)
                        wout_pending.remove(c)

            # ---- W_out (sel folded into straight/flipped weights on host) ----
            for c in wout_pending:
                emit_wout(c)
            wffn = scanp.tile([128, FBF], BF16, tag="bc", name="wffn")
            nc.sync.dma_start(out=wffn, in_=blob_ffn[:, :])
            nc.gpsimd.collective_compute(
                "ReduceScatter", OP.add,
                replica_groups=[[0, 1, 2, 3], [4, 5, 6, 7]],
                ins=[rs_in[:, :]], outs=[rs_out[:, :]])

            # ---- FFN + layernorm on the 1024-token slab ----
            ysb_all = persist.tile([128, 8 * DM], FP32, tag="xtp0", name="ysba")
            ro = rs_out[0:128, :]
            nc.sync.dma_start(
                out=ysb_all,
                in_=bass.AP(tensor=ro.tensor, offset=ro.offset,
                            ap=[ro.ap[0], [128 * DM, 8]] + ro.ap[1:]))
            ysb_bf = persist.tile([128, 8 * DM], BF16, tag="xtp1", name="ysbb")
            ysb = [ysb_bf[:, t8 * DM:(t8 + 1) * DM] for t8 in range(8)]
            ysbf = [ysb_all[:, t8 * DM:(t8 + 1) * DM] for t8 in range(8)]
            yT_bf = [hqp.tile([128, T_SLAB * 2], BF16, tag=f"hq{k}",
                              name=f"yT{k}")[:, 0:T_SLAB] for k in range(2)]
            nc.vector.tensor_copy(ysb_bf, ysb_all)
            for t8 in range(8):
                for k in range(2):
                    pst = psum.tile([128, CJ * 2], BF16, tag="mm",
                                    name="mmtr")[:, 0:128]
                    nc.tensor.transpose(pst, ysb[t8][:, k * 128:(k + 1) * 128],
                                        identity_bf)
                    nc.scalar.copy(yT_bf[k][:, t8 * 128:(t8 + 1) * 128], pst)
            # gelu tiles alias the yfin slabs (dead after ysel)
            g_sb = [yfin[m // 4][:, (m % 4) * T_SLAB:(m % 4 + 1) * T_SLAB]
                    for m in range(8)]
            for m in range(8):
                for jj in range(2):
                    ps = psum.tile([128, CJ], FP32, tag="mm", name="mmf")
                    for k in range(2):
                        nc.tensor.matmul(
                            ps,
                            wffn[:, F1_OFF + k * H_FF + m * 128:
                                 F1_OFF + k * H_FF + (m + 1) * 128],
                            yT_bf[k][:, jj * 512:(jj + 1) * 512],
                            start=(k == 0), stop=(k == 1))
                    nc.scalar.activation(g_sb[m][:, jj * 512:(jj + 1) * 512], ps,
                                         AF.Gelu,
                                         bias=wf[:, BF1_OFF + m:BF1_OFF + m + 1])
            for t8 in range(8):
                tsl = slice(t8 * 128, (t8 + 1) * 128)
                ps = psum.tile([128, CJ], FP32, tag="mm", name="mm")[:, 0:DM]
                for m in range(8):
                    nc.tensor.matmul(ps, g_sb[m][:, tsl],
                                     wffn[:, F2_OFF + m * DM:F2_OFF + (m + 1) * DM],
                                     start=(m == 0), stop=(m == 7))
                ff = work.tile([128, DM], FP32, tag="ff", name="ff", bufs=2)
                nc.vector.tensor_tensor(ff, ps, wf[:, BF2_OFF:BF2_OFF + DM], OP.add)
                stats = work.tile([128, 6], FP32, tag="stats", name="stats", bufs=2)
                nc.vector.bn_stats(stats, ff)
                mv = work.tile([128, 2], FP32, tag="mv", name="mv", bufs=2)
                nc.vector.bn_aggr(mv, stats)
                sq = work.tile([128, 1], FP32, tag="sq", name="sq", bufs=2)
                nc.scalar.activation(sq, mv[:, 1:2], AF.Sqrt, bias=eps_sb[:, 0:1])
                rstd = work.tile([128, 1], FP32, tag="rstd", name="rstd", bufs=2)
                nc.vector.reciprocal(rstd, sq)
                norm = work.tile([128, DM], FP32, tag="norm", name="norm", bufs=2)
                nc.vector.tensor_scalar(norm, ff, mv[:, 0:1], rstd,
                                        OP.subtract, OP.mult)
                nc.vector.tensor_tensor(norm, norm, wf[:, G2_OFF:G2_OFF + DM],
                                        OP.mult)
                nc.vector.tensor_tensor(norm, norm, wf[:, BE2_OFF:BE2_OFF + DM],
                                        OP.add)
                nc.vector.tensor_tensor(norm, norm, ysbf[t8], OP.add)
                nc.sync.dma_start(out=out_slab[tsl, :], in_=norm)

    nc.compile()
    return nc


def _prep_inputs(inputs):
    x = np.asarray(inputs["x"], np.float32)
    W_in = np.asarray(inputs["W_in"], np.float32)
    conv_w = np.asarray(inputs["conv_w"], np.float32)
    conv_b = np.asarray(inputs["conv_b"], np.float32)
    W_xproj = np.asarray(inputs["W_xproj"], np.float32)
    W_dt = np.asarray(inputs["W_dt"], np.float32)
    b_dt = np.asarray(inputs["b_dt"], np.float32)
    A_log = np.asarray(inputs["A_log"], np.float32)
    Dp = np.asarray(inputs["Dp"], np.float32)
    W_out = np.asarray(inputs["W_out"], np.float32)
    W_ff1 = np.asarray(inputs["W_ff1"], np.float32)
    b_ff1 = np.asarray(inputs["b_ff1"], np.float32)
    W_ff2 = np.asarray(inputs["W_ff2"], np.float32)
    b_ff2 = np.asarray(inputs["b_ff2"], np.float32)
    g2 = np.asarray(inputs["g2"], np.float32)
    beta2 = np.asarray(inputs["beta2"], np.float32)

    A = -np.exp(A_log)  # (512, 16)

    # gfold: G_k[m, d] = W_in[m, d] * conv_w[d, k]   (xs columns, both halves)
    W_xs = W_in[:, :DI_FULL]
    gf = np.zeros((128, GF_SZ), np.float32)
    for p in range(2):
        for k in range(DC):
            for db in range(4):
                off = p * (DC * 4 * 128) + k * (4 * 128) + db * 128
                gf[:, off:off + 128] = (
                    W_xs[p * 128:(p + 1) * 128, db * 128:(db + 1) * 128]
                    * conv_w[db * 128:(db + 1) * 128, k][None, :])

    in_maps = []
    for c in range(8):
        seq, half = c // 2, c % 2
        bb, bwd = seq // 2, seq % 2
        xs_np = x[bb]
        if bwd:
            xs_np = xs_np[::-1]
        xpad = np.zeros((DM, LP), np.float32)
        xpad[:, DC - 1:] = xs_np.T
        hsl = slice(half * DI, (half + 1) * DI)

        # d-block order: own half's two blocks first (the kernel scans
        # blocks 0,1 with the per-half parameters)
        dbo = [0, 1, 2, 3] if half == 0 else [2, 3, 0, 1]
        blob = np.zeros((128, FB), np.float32)
        for p in range(2):
            for k in range(DC):
                for dbi, db in enumerate(dbo):
                    off_src = p * (DC * 4 * 128) + k * (4 * 128) + db * 128
                    off_dst = p * (DC * 4 * 128) + k * (4 * 128) + dbi * 128
                    blob[:, GF_OFF + off_dst:GF_OFF + off_dst + 128] = \
                        gf[:, off_src:off_src + 128]
        for p in range(2):
            blob[:, WZ_OFF + p * DI:WZ_OFF + (p + 1) * DI] = \
                W_in[p * 128:(p + 1) * 128,
                     DI_FULL + half * DI:DI_FULL + (half + 1) * DI]
        # B/C columns pair-grouped to match the K_SEQ processing order
        perm = list(range(RK))
        for p in range(NS // 2):
            n0, n1 = K_SEQ[2 * p] - 1, K_SEQ[2 * p + 1] - 1
            perm += [RK + n0, RK + n1, RK + NS + n0, RK + NS + n1]
        W_xp_perm = W_xproj[:, perm]
        for dbi, db in enumerate(dbo):
            blob[:, XP_OFF + dbi * PJ:XP_OFF + (dbi + 1) * PJ] = \
                W_xp_perm[db * 128:(db + 1) * 128, :]
        blob[0:RK, DT_OFF:DT_OFF + DI] = W_dt[:, hsl]
        for b2 in range(2):
            blob[:, WO_OFF + b2 * DM:WO_OFF + (b2 + 1) * DM] = \
                W_out[half * DI + b2 * 128:half * DI + (b2 + 1) * 128, :]
        fb = np.zeros((128, FBF), np.float32)
        for p in range(2):
            fb[:, F1_OFF + p * H_FF:F1_OFF + (p + 1) * H_FF] = \
                W_ff1[p * 128:(p + 1) * 128, :]
        for m in range(8):
            fb[:, F2_OFF + m * DM:F2_OFF + (m + 1) * DM] = \
                W_ff2[m * 128:(m + 1) * 128, :]

        fblob = np.zeros((128, FF), np.float32)
        for b2 in range(2):
            fblob[:, A_OFF + b2 * NS:A_OFF + (b2 + 1) * NS] = \
                A[half * DI + b2 * 128:half * DI + (b2 + 1) * 128, :]
            fblob[:, BD_OFF + b2] = b_dt[half * DI + b2 * 128:
                                         half * DI + (b2 + 1) * 128]
            fblob[:, DP_OFF + b2] = Dp[half * DI + b2 * 128:
                                       half * DI + (b2 + 1) * 128]
        for dbi, db in enumerate(dbo):
            fblob[:, CB_OFF + dbi] = conv_b[db * 128:(db + 1) * 128]
        fblob[:, BF1_OFF:BF1_OFF + 8] = b_ff1.reshape(8, 128).T
        fblob[:, BF2_OFF:BF2_OFF + DM] = b_ff2[None, :]
        fblob[:, G2_OFF:G2_OFF + DM] = g2[None, :]
        fblob[:, BE2_OFF:BE2_OFF + DM] = beta2[None, :]
        fblob[:, SEL_OFF] = 1.0 - bwd
        fblob[:, SEL_OFF + 1] = float(bwd)

        in_maps.append(dict(
            xT=xpad.astype(BF),
            blob_bf=blob.astype(BF),
            blob_ffn=fb.astype(BF),
            blob_f32=fblob,
        ))
    return in_maps


def kernel(**inputs) -> np.ndarray:
    if "nc" not in _NC_CACHE:
        _NC_CACHE["nc"] = build_program()
    nc = _NC_CACHE["nc"]
    in_maps = _prep_inputs(inputs)
    res = run_bass_kernel_spmd(nc, in_maps, core_ids=list(range(8)))
    out = np.empty((B, L, DM), np.float32)
    for c in range(8):
        bb = c // 4
        r = c % 4
        out[bb, r * T_SLAB:(r + 1) * T_SLAB, :] = res.results[c]["out_slab"]
    return out


# revision 35
# speedup vs baseline: 2.6641x; 1.3353x over previous
"""BiMambaBlock Trainium2 kernel (8-core SPMD), v2.

Sharding: core c -> (seq = c//2, half = c%2); seq in {b0 fwd, b0 bwd,
b1 fwd, b1 bwd}, half = 256 of the 512 d_inner channels.

v2 changes vs baseline:
 - No xproj AllReduce: every core computes xc for BOTH halves (the
   depthwise conv folds into 4 time-shifted PE matmuls with
   G_k = W_in ⊙ conv_w[:,k]), so the (48, L) projection is local.
 - Scan phase rebalanced: scans + fused scalar_tensor_tensor multiplies
   (4x DVE mode, bf16) on DVE, a tuned share of plain multiplies on
   GPSIMD, dA = exp(A_n δ) on Act, lane accumulation as pair-packed
   software-DGE accumulate DMAs.
 - Act tables batched per quarter (silu | exp+ln+exp) to avoid thrash.
 - Weights arrive as two packed SBUF blobs (2 DMAs), x as 2 padded DMAs.
 - ReduceScatter runs in bf16.
"""
import sys

for _p in ("/opt/trn_rl_repo",):
    if _p not in sys.path:
        sys.path.insert(0, _p)

import numpy as np
import ml_dtypes
from contextlib import ExitStack

import concourse.bass as bass
import concourse.tile as tile
from concourse import bacc, mybir
from concourse.bass_utils import run_bass_kernel_spmd
from concourse.masks import make_identity

BF = ml_dtypes.bfloat16
FP32 = mybir.dt.float32
BF16 = mybir.dt.bfloat16
AF = mybir.ActivationFunctionType
OP = mybir.AluOpType

# problem constants
B, L, DM = 2, 4096, 256
DI_FULL = 512
DI = 256            # own-half channels
NS = 16
RK = 16
DC = 4
H_FF = 1024
T_SLAB = 1024
NQ = 4
LQ = L // NQ        # 1024
LN_EPS = 1e-5
PJ = RK + 2 * NS    # 48
CJ = 512            # lead-in chunk
LP = L + DC - 1     # padded x length (4099)

# blob_bf column layout
GF_OFF = 0                       # gfold: pblk(2) x k(4) x db(4) x 128
GF_SZ = 2 * DC * 4 * 128         # 4096
WZ_OFF = GF_OFF + GF_SZ          # w_z: pblk(2) x 256
WZ_SZ = 2 * DI
XP_OFF = WZ_OFF + WZ_SZ          # w_xproj: db(4) x 48
XP_SZ = 4 * PJ
DT_OFF = XP_OFF + XP_SZ          # w_dt: [16 rows used] x 256
DT_SZ = DI
WO_OFF = DT_OFF + DT_SZ          # w_out: variant(2: straight/flip) x b2(2) x 256
WO_SZ = 4 * DM
FB = WO_OFF + WO_SZ
# blob_ffn column layout (loaded post-scan into a recycled scan buffer)
F1_OFF = 0                       # w_ff1: pblk(2) x 1024
F1_SZ = 2 * H_FF
F2_OFF = F1_OFF + F1_SZ          # w_ff2: m(8) x 256
F2_SZ = 8 * DM
FBF = F2_OFF + F2_SZ

# blob_f32 column layout
A_OFF = 0                        # A cols: b2(2) x 16
BD_OFF = A_OFF + 2 * NS          # b_dt: b2(2)
DP_OFF = BD_OFF + 2              # Dp: b2(2)
CB_OFF = DP_OFF + 2              # conv_b: db(4)
BF1_OFF = CB_OFF + 4             # b_ff1: 8
BF2_OFF = BF1_OFF + 8            # b_ff2 row-bcast: 256
G2_OFF = BF2_OFF + DM            # g2 row-bcast: 256
BE2_OFF = G2_OFF + DM            # beta2 row-bcast: 256
SEL_OFF = BE2_OFF + DM           # sel: 2
FF = SEL_OFF + 2

# scan state processing order (k = n+1); squares ride any act table.
# pairs of consecutive positions share one B/C broadcast DMA.
K_SEQ = [8, 16, 1, 2, 4, 3, 6, 12, 5, 10, 7, 14, 9, 11, 13, 15]
SQ_SRC = {16: 8, 2: 1, 4: 2, 6: 3, 12: 6, 10: 5, 14: 7}   # k -> source k

_NC_CACHE = {}


def _gcol(pblk, k, db):
    return GF_OFF + pblk * (DC * 4 * 128) + k * (4 * 128) + db * 128


def _inp(nc, name, shape, dtype):
    return nc.dram_tensor(name, shape, dtype, kind="ExternalInput").ap()


def _out(nc, name, shape, dtype):
    return nc.dram_tensor(name, shape, dtype, kind="ExternalOutput").ap()


def _bcast(row_ap, parts=128):
    """AP replicating a DRAM row-pair slice across `parts` partitions."""
    return bass.AP(tensor=row_ap.tensor, offset=row_ap.offset,
                   ap=[[0, parts]] + row_ap.ap)


class _Split:
    def __init__(self, frac):
        self.frac = frac
        self.acc = 0.0

    def pool(self):
        self.acc += self.frac
        if self.acc >= 1.0:
            self.acc -= 1.0
            return True
        return False


def build_program(reps=1):
    nc = bacc.Bacc("TRN2", target_bir_lowering=False, debug=False, num_devices=8)

    xT = _inp(nc, "xT", [2 * 128, LP], BF16)          # padded, flipped if bwd
    blob_bf = _inp(nc, "blob_bf", [128, FB], BF16)
    blob_ffn = _inp(nc, "blob_ffn", [128, FBF], BF16)
    blob_f32 = _inp(nc, "blob_f32", [128, FF], FP32)
    out_slab = _out(nc, "out_slab", [T_SLAB, DM], FP32)

    bc_dram = nc.dram_tensor("bc_dram", [2 * NS, L], BF16)
    rs_in = nc.dram_tensor("rs_in", [L, DM], FP32)
    rs_out = nc.dram_tensor("rs_out", [T_SLAB, DM], FP32)

    with tile.TileContext(nc) as tc, ExitStack() as ctx:
        consts = ctx.enter_context(tc.tile_pool(name="consts", bufs=1))
        persist = ctx.enter_context(tc.tile_pool(name="persist", bufs=1))
        qpool = ctx.enter_context(tc.tile_pool(name="qpool", bufs=2))
        work = ctx.enter_context(tc.tile_pool(name="work", bufs=1))
        scanp = ctx.enter_context(tc.tile_pool(name="scanp", bufs=3))
        hqp = ctx.enter_context(tc.tile_pool(name="hqp", bufs=2))
        psum = ctx.enter_context(tc.tile_pool(name="psum", bufs=8, space="PSUM"))

        wb = consts.tile([128, FB], BF16, tag="wb", name="wb")
        nc.sync.dma_start(out=wb, in_=blob_bf[:, :])
        wf = consts.tile([128, FF], FP32, tag="wf", name="wf")
        nc.sync.dma_start(out=wf, in_=blob_f32[:, :])
        eps_sb = consts.tile([128, 1], FP32, tag="eps", name="eps")
        nc.vector.memset(eps_sb, LN_EPS)
        identity_bf = consts.tile([128, 128], BF16, tag="idn", name="idn")
        make_identity(nc, identity_bf)

        for _rep in range(reps):
            xTp = [persist.tile([128, LP], BF16, tag=f"xtp{p}", name=f"xtp{p}")
                   for p in range(2)]
            for p in range(2):
                nc.sync.dma_start(out=xTp[p], in_=xT[p * 128:(p + 1) * 128, :])

            yfin = [persist.tile([128, L], BF16, tag=f"yf{b}", name=f"yf{b}")
                    for b in range(2)]
            lanes = [persist.tile([128, 4 * LQ], BF16, tag=f"lq{b}", name=f"lq{b}")
                     for b in range(2)]
            h_last = persist.tile([128, 2 * NS], FP32, tag="hl", name="hl")

            def mul_tt(out, a, b2_):
                nc.vector.tensor_tensor(out, a, b2_, OP.mult)

            def add_tt(out, a, b2_):
                nc.vector.tensor_tensor(out, a, b2_, OP.add)

            # per-quarter rotating slab state (filled by emit_leadin)
            slabs = {}

            def emit_leadin(q):
                """xc/z/proj/dt for quarter q (chunks 2q, 2q+1)."""
                qsl = slice(q * LQ, (q + 1) * LQ)
                xc_q = [qpool.tile([128, LQ], BF16, tag=f"xcq{b}", name=f"xcq{b}")
                        for b in range(2)]
                z_q = [qpool.tile([128, LQ], BF16, tag=f"zq{b}", name=f"zq{b}")
                       for b in range(2)]
                m_q = [qpool.tile([128, LQ], BF16, tag=f"mq{b}", name=f"mq{b}",
                                  bufs=1) for b in range(2)]
                bc_q = qpool.tile([PJ, LQ], BF16, tag="bcq", name="bcq", bufs=1)
                for jj in range(2):
                    j = 2 * q + jj
                    csl = slice(jj * CJ, (jj + 1) * CJ)
                    xc_ch = []
                    for db in range(4):
                        ps = psum.tile([128, CJ], FP32, tag="mm", name="mm")
                        first = True
                        for p in range(2):
                            for k in range(DC):
                                nc.tensor.matmul(
                                    ps, wb[:, _gcol(p, k, db):_gcol(p, k, db) + 128],
                                    xTp[p][:, j * CJ + k:j * CJ + k + CJ],
                                    start=first, stop=(p == 1 and k == DC - 1))
                                first = False
                        if db < 2:
                            dst = xc_q[db][:, csl]
                        else:
                            dst = work.tile([128, CJ], BF16, tag=f"xco{db}",
                                            name=f"xco{db}", bufs=2)
                        nc.scalar.activation(dst, ps, AF.Silu,
                                             bias=wf[:, CB_OFF + db:CB_OFF + db + 1])
                        xc_ch.append(dst)
                    for b2 in range(2):
                        ps = psum.tile([128, CJ], FP32, tag="mm", name="mm")
                        for p in range(2):
                            nc.tensor.matmul(
                                ps, wb[:, WZ_OFF + p * DI + b2 * 128:
                                       WZ_OFF + p * DI + b2 * 128 + 128],
                                xTp[p][:, DC - 1 + j * CJ:DC - 1 + j * CJ + CJ],
                                start=(p == 0), stop=(p == 1))
                        nc.scalar.activation(z_q[b2][:, csl], ps, AF.Silu)
                    # xproj (full 512 channels, local)
                    ps = psum.tile([128, CJ], FP32, tag="mm", name="mm")[0:PJ, :]
                    for db in range(4):
                        nc.tensor.matmul(ps, wb[:, XP_OFF + db * PJ:
                                                XP_OFF + (db + 1) * PJ],
                                         xc_ch[db], start=(db == 0), stop=(db == 3))
                    nc.scalar.copy(bc_q[:, csl], ps)
                    # dt matmul -> m staging (exp/ln batched later)
                    for b2 in range(2):
                        ps = psum.tile([128, CJ], FP32, tag="mm", name="mm")
                        nc.tensor.matmul(ps, wb[0:16, DT_OFF + b2 * 128:
                                                DT_OFF + b2 * 128 + 128],
                                         bc_q[0:RK, csl], start=True, stop=True)
                        nc.scalar.copy(m_q[b2][:, csl], ps)
                # stage B/C rows to DRAM for partition-broadcast reads
                nc.sync.dma_start(out=bc_dram[:, qsl], in_=bc_q[RK:PJ, :])
                slabs[q] = dict(xc=xc_q, z=z_q, m=m_q)

            def emit_delta(q):
                """delta = ln(1 + exp(m + b_dt)), dx = delta*xc. Emits the
                two Exp's (same act table as the scan's dA exps), then Ln."""
                s = slabs[q]
                d_q = [qpool.tile([128, LQ], BF16, tag=f"dq{b}", name=f"dq{b}")
                       for b in range(2)]
                dx_q = [qpool.tile([128, LQ], BF16, tag=f"dxq{b}", name=f"dxq{b}")
                        for b in range(2)]
                eus = []
                for b2 in range(2):
                    eu = work.tile([128, LQ], BF16, tag="eu", name="eu", bufs=1)
                    nc.scalar.activation(eu, s["m"][b2], AF.Exp,
                                         bias=wf[:, BD_OFF + b2:BD_OFF + b2 + 1])
                    eus.append(eu)
                for b2 in range(2):
                    nc.scalar.activation(d_q[b2], eus[b2], AF.Ln, bias=1.0)
                    mul_tt(dx_q[b2], d_q[b2], s["xc"][b2])
                s["d"] = d_q
                s["dx"] = dx_q

            ysel_t = {}

            def emit_wout(c):
                """ysel + rs_in rows for token chunk c (sel applied on DVE;
                flip reads yfin mirrored)."""
                csl = slice(c * LQ, (c + 1) * LQ)
                fsl = slice(L - (c + 1) * LQ, L - c * LQ)
                for b2 in range(2):
                    if b2 not in ysel_t:
                        ysel_t[b2] = persist.tile([128, L], BF16,
                                                  tag=f"xtp{b2}",
                                                  name=f"ysel{b2}")
                    ys = ysel_t[b2]
                    t = work.tile([128, LQ], BF16, tag="s1", name="ysl0",
                                  bufs=1)
                    nc.vector.tensor_scalar(t, yfin[b2][:, csl],
                                            wf[:, SEL_OFF:SEL_OFF + 1],
                                            None, OP.mult)
                    tf = work.tile([128, LQ], BF16, tag="s2", name="ysl1",
                                   bufs=1)
                    nc.vector.tensor_scalar(tf, yfin[b2][:, fsl][:, ::-1],
                                            wf[:, SEL_OFF + 1:SEL_OFF + 2],
                                            None, OP.mult)
                    add_tt(ys[:, csl], t, tf)
                for t8 in range(c * 8, (c + 1) * 8):
                    tsl = slice(t8 * 128, (t8 + 1) * 128)
                    ps = psum.tile([128, CJ], FP32, tag="mm", name="mm")[:, 0:DM]
                    for b2 in range(2):
                        nc.tensor.matmul(
                            ps, ysel_t[b2][:, tsl],
                            wb[:, WO_OFF + b2 * DM:WO_OFF + (b2 + 1) * DM],
                            start=(b2 == 0), stop=(b2 == 1))
                    yp = work.tile([128, DM], FP32, tag="ypart", name="ypart",
                                   bufs=3)
                    nc.scalar.copy(yp, ps)
                    nc.sync.dma_start(out=rs_in[tsl, :], in_=yp)

            emit_leadin(0)
            emit_delta(0)
            wout_pending = [0, 1, 2, 3]

            for q in range(NQ):
                qsl = slice(q * LQ, (q + 1) * LQ)
                s = slabs.pop(q)
                xc_q, z_q, d_q, dx_q = s["xc"], s["z"], s["d"], s["dx"]
                # ---- selective scan over 16 states (B/C fetched in pairs,
                # processed in K_SEQ order; even decays from Act Square) ----
                hq_cur = [None, None]
                hc_split = _Split(0.81)    # share of hc multiplies on GPSIMD
                dA_prev = [{}, {}]         # k -> dA tile (square sources)
                for pos in range(NS):
                    n = K_SEQ[pos] - 1
                    if pos % 2 == 0:
                        bc = scanp.tile([128, 4 * LQ], BF16, tag="bc", name="bc",
                                        bufs=3)
                        # rows 4p..4p+3 = pair-grouped (host-permuted to K_SEQ)
                        row = bc_dram[2 * pos:2 * pos + 1, qsl]
                        nc.sync.dma_start(
                            out=bc,
                            in_=bass.AP(tensor=row.tensor, offset=row.offset,
                                        ap=[[0, 128], [L, 4]] + row.ap[1:]))
                    bco = (pos % 2) * LQ
                    for b2 in range(2):
                        hcol = NS * b2 + n
                        dA = scanp.tile([128, LQ], BF16, tag=f"dA{b2}",
                                        name=f"dA{b2}", bufs=4)
                        k = n + 1
                        if k in SQ_SRC:
                            nc.scalar.activation(dA, dA_prev[b2][SQ_SRC[k]],
                                                 AF.Square)
                        else:
                            nc.scalar.activation(
                                dA, d_q[b2], AF.Exp,
                                scale=wf[:, A_OFF + b2 * NS + n:
                                         A_OFF + b2 * NS + n + 1])
                        dA_prev[b2][k] = dA
                        dBx = scanp.tile([128, LQ], BF16, tag=f"dBx{b2}",
                                         name=f"dBx{b2}", bufs=3)
                        nc.vector.tensor_tensor(dBx, dx_q[b2], bc[:, bco:bco + LQ],
                                                OP.mult)
                        h = scanp.tile([128, LQ], BF16, tag=f"h{b2}", name=f"h{b2}")
                        init = 0.0 if q == 0 else h_last[:, hcol:hcol + 1]
                        nc.vector.tensor_tensor_scan(h, dA, dBx, init,
                                                     OP.mult, OP.add)
                        if q < NQ - 1:
                            nc.vector.tensor_copy(h_last[:, hcol:hcol + 1],
                                                  h[:, LQ - 1:LQ])
                        hc_eng = (nc.gpsimd if hc_split.pool() else nc.vector)
                        if pos < 4:
                            hc_eng.tensor_tensor(
                                lanes[b2][:, pos * LQ:(pos + 1) * LQ],
                                h, bc[:, 2 * LQ + bco:2 * LQ + bco + LQ], OP.mult)
                        else:
                            slot = (pos - 4) % 2
                            if slot == 0:
                                hq_cur[b2] = hqp.tile([128, 2 * LQ], BF16,
                                                      tag=f"hq{b2}",
                                                      name=f"hq{b2}")
                            hc_eng.tensor_tensor(
                                hq_cur[b2][:, slot * LQ:(slot + 1) * LQ],
                                h, bc[:, 2 * LQ + bco:2 * LQ + bco + LQ], OP.mult)
                            if slot == 1:
                                base = (pos - 5) % 4
                                nc.gpsimd.dma_start(
                                    out=lanes[b2][:, base * LQ:(base + 2) * LQ],
                                    in_=hq_cur[b2], accum_op=OP.add)
                # next quarter's lead-in + exp/ln emitted after the scan so the
                # act-queue batches by table (exps | silus | exp+ln)
                if q + 1 < NQ:
                    emit_leadin(q + 1)
                    emit_delta(q + 1)
                # ---- finish quarter: y = (xc*Dp + sum lanes) * silu(z) ----
                for b2 in range(2):
                    s1 = work.tile([128, LQ], BF16, tag="s1", name="s1", bufs=1)
                    add_tt(s1, lanes[b2][:, 0:LQ], lanes[b2][:, LQ:2 * LQ])
                    s2 = work.tile([128, LQ], BF16, tag="s2", name="s2", bufs=1)
                    add_tt(s2, lanes[b2][:, 2 * LQ:3 * LQ],
                           lanes[b2][:, 3 * LQ:4 * LQ])
                    xcdp = work.tile([128, LQ], BF16, tag="t1", name="xcdp", bufs=1)
                    nc.vector.tensor_scalar(xcdp, xc_q[b2],
                                            wf[:, DP_OFF + b2:DP_OFF + b2 + 1],
                                            None, OP.mult)
                    t1 = work.tile([128, LQ], BF16, tag="t1b", name="t1b", bufs=1)
                    add_tt(t1, xcdp, s1)
                    t2 = work.tile([128, LQ], BF16, tag="t2", name="t2", bufs=1)
                    add_tt(t2, t1, s2)
                    mul_tt(yfin[b2][:, qsl], t2, z_q[b2])
                if q == 2:
                    # middle output chunks need only yfin quarters 1,2
                    for c in (1, 2):
                        emit_wout(c)
                        wout_pending.remove(c)

            # ---- W_out (sel folded into straight/flipped weights on host) ----
            for c in wout_pending:
                emit_wout(c)
            wffn = scanp.tile([128, FBF], BF16, tag="bc", name="wffn")
            nc.sync.dma_start(out=wffn, in_=blob_ffn[:, :])
            nc.gpsimd.collective_compute(
                "ReduceScatter", OP.add,
                replica_groups=[[0, 1, 2, 3], [4, 5, 6, 7]],
                ins=[rs_in[:, :]], outs=[rs_out[:, :]])

            # ---- FFN + layernorm on the 1024-token slab ----
            ysb_all = persist.tile([128, 8 * DM], FP32, tag="xtp0", name="ysba")
            ro = rs_out[0:128, :]
            nc.sync.dma_start(
                out=ysb_all,
                in_=bass.AP(tensor=ro.tensor, offset=ro.offset,
                            ap=[ro.ap[0], [128 * DM, 8]] + ro.ap[1:]))
            ysb_bf = persist.tile([128, 8 * DM], BF16, tag="xtp1", name="ysbb")
            ysb = [ysb_bf[:, t8 * DM:(t8 + 1) * DM] for t8 in range(8)]
            ysbf = [ysb_all[:, t8 * DM:(t8 + 1) * DM] for t8 in range(8)]
            yT_bf = [hqp.tile([128, T_SLAB * 2], BF16, tag=f"hq{k}",
                              name=f"yT{k}")[:, 0:T_SLAB] for k in range(2)]
            nc.vector.tensor_copy(ysb_bf, ysb_all)
            for t8 in range(8):
                for k in range(2):
                    pst = psum.tile([128, CJ * 2], BF16, tag="mm",
                                    name="mmtr")[:, 0:128]
                    nc.tensor.transpose(pst, ysb[t8][:, k * 128:(k + 1) * 128],
                                        identity_bf)
                    nc.scalar.copy(yT_bf[k][:, t8 * 128:(t8 + 1) * 128], pst)
            # gelu tiles alias the yfin slabs (dead after ysel)
            g_sb = [yfin[m // 4][:, (m % 4) * T_SLAB:(m % 4 + 1) * T_SLAB]
                    for m in range(8)]
            for m in range(8):
                for jj in range(2):
                    ps = psum.tile([128, CJ], FP32, tag="mm", name="mmf")
                    for k in range(2):
                        nc.tensor.matmul(
                            ps,
                            wffn[:, F1_OFF + k * H_FF + m * 128:
                                 F1_OFF + k * H_FF + (m + 1) * 128],
                            yT_bf[k][:, jj * 512:(jj + 1) * 512],
                            start=(k == 0), stop=(k == 1))
                    nc.scalar.activation(g_sb[m][:, jj * 512:(jj + 1) * 512], ps,
                                         AF.Gelu,
                                         bias=wf[:, BF1_OFF + m:BF1_OFF + m + 1])
            for t8 in range(8):
                tsl = slice(t8 * 128, (t8 + 1) * 128)
                ps = psum.tile([128, CJ], FP32, tag="mm", name="mm")[:, 0:DM]
                for m in range(8):
                    nc.tensor.matmul(ps, g_sb[m][:, tsl],
                                     wffn[:, F2_OFF + m * DM:F2_OFF + (m + 1) * DM],
                                     start=(m == 0), stop=(m == 7))
                ff = work.tile([128, DM], FP32, tag="ff", name="ff", bufs=2)
                nc.vector.tensor_tensor(ff, ps, wf[:, BF2_OFF:BF2_OFF + DM], OP.add)
                stats = work.tile([128, 6], FP32, tag="stats", name="stats", bufs=2)
                nc.vector.bn_stats(stats, ff)
                mv = work.tile([128, 2], FP32, tag="mv", name="mv", bufs=2)
                nc.vector.bn_aggr(mv, stats)
                sq = work.tile([128, 1], FP32, tag="sq", name="sq", bufs=2)
                nc.scalar.activation(sq, mv[:, 1:2], AF.Sqrt, bias=eps_sb[:, 0:1])
                rstd = work.tile([128, 1], FP32, tag="rstd", name="rstd", bufs=2)
                nc.vector.reciprocal(rstd, sq)
                norm = work.tile([128, DM], FP32, tag="norm", name="norm", bufs=2)
                nc.vector.tensor_scalar(norm, ff, mv[:, 0:1], rstd,
                                        OP.subtract, OP.mult)
                nc.vector.tensor_tensor(norm, norm, wf[:, G2_OFF:G2_OFF + DM],
                                        OP.mult)
                nc.vector.tensor_tensor(norm, norm, wf[:, BE2_OFF:BE2_OFF + DM],
                                        OP.add)
                nc.vector.tensor_tensor(norm, norm, ysbf[t8], OP.add)
                nc.sync.dma_start(out=out_slab[tsl, :], in_=norm)

    nc.compile()
    return nc


def _prep_inputs(inputs):
    x = np.asarray(inputs["x"], np.float32)
    W_in = np.asarray(inputs["W_in"], np.float32)
    conv_w = np.asarray(inputs["conv_w"], np.float32)
    conv_b = np.asarray(inputs["conv_b"], np.float32)
    W_xproj = np.asarray(inputs["W_xproj"], np.float32)
    W_dt = np.asarray(inputs["W_dt"], np.float32)
    b_dt = np.asarray(inputs["b_dt"], np.float32)
    A_log = np.asarray(inputs["A_log"], np.float32)
    Dp = np.asarray(inputs["Dp"], np.float32)
    W_out = np.asarray(inputs["W_out"], np.float32)
    W_ff1 = np.asarray(inputs["W_ff1"], np.float32)
    b_ff1 = np.asarray(inputs["b_ff1"], np.float32)
    W_ff2 = np.asarray(inputs["W_ff2"], np.float32)
    b_ff2 = np.asarray(inputs["b_ff2"], np.float32)
    g2 = np.asarray(inputs["g2"], np.float32)
    beta2 = np.asarray(inputs["beta2"], np.float32)

    A = -np.exp(A_log)  # (512, 16)

    # gfold: G_k[m, d] = W_in[m, d] * conv_w[d, k]   (xs columns, both halves)
    W_xs = W_in[:, :DI_FULL]
    gf = np.zeros((128, GF_SZ), np.float32)
    for p in range(2):
        for k in range(DC):
            for db in range(4):
                off = p * (DC * 4 * 128) + k * (4 * 128) + db * 128
                gf[:, off:off + 128] = (
                    W_xs[p * 128:(p + 1) * 128, db * 128:(db + 1) * 128]
                    * conv_w[db * 128:(db + 1) * 128, k][None, :])

    in_maps = []
    for c in range(8):
        seq, half = c // 2, c % 2
        bb, bwd = seq // 2, seq % 2
        xs_np = x[bb]
        if bwd:
            xs_np = xs_np[::-1]
        xpad = np.zeros((DM, LP), np.float32)
        xpad[:, DC - 1:] = xs_np.T
        hsl = slice(half * DI, (half + 1) * DI)

        # d-block order: own half's two blocks first (the kernel scans
        # blocks 0,1 with the per-half parameters)
        dbo = [0, 1, 2, 3] if half == 0 else [2, 3, 0, 1]
        blob = np.zeros((128, FB), np.float32)
        for p in range(2):
            for k in range(DC):
                for dbi, db in enumerate(dbo):
                    off_src = p * (DC * 4 * 128) + k * (4 * 128) + db * 128
                    off_dst = p * (DC * 4 * 128) + k * (4 * 128) + dbi * 128
                    blob[:, GF_OFF + off_dst:GF_OFF + off_dst + 128] = \
                        gf[:, off_src:off_src + 128]
        for p in range(2):
            blob[:, WZ_OFF + p * DI:WZ_OFF + (p + 1) * DI] = \
                W_in[p * 128:(p + 1) * 128,
                     DI_FULL + half * DI:DI_FULL + (half + 1) * DI]
        # B/C columns pair-grouped to match the K_SEQ processing order
        perm = list(range(RK))
        for p in range(NS // 2):
            n0, n1 = K_SEQ[2 * p] - 1, K_SEQ[2 * p + 1] - 1
            perm += [RK + n0, RK + n1, RK + NS + n0, RK + NS + n1]
        W_xp_perm = W_xproj[:, perm]
        for dbi, db in enumerate(dbo):
            blob[:, XP_OFF + dbi * PJ:XP_OFF + (dbi + 1) * PJ] = \
                W_xp_perm[db * 128:(db + 1) * 128, :]
        blob[0:RK, DT_OFF:DT_OFF + DI] = W_dt[:, hsl]
        for b2 in range(2):
            blob[:, WO_OFF + b2 * DM:WO_OFF + (b2 + 1) * DM] = \
                W_out[half * DI + b2 * 128:half * DI + (b2 + 1) * 128, :]
        fb = np.zeros((128, FBF), np.float32)
        for p in range(2):
            fb[:, F1_OFF + p * H_FF:F1_OFF + (p + 1) * H_FF] = \
                W_ff1[p * 128:(p + 1) * 128, :]
        for m in range(8):
            fb[:, F2_OFF + m * DM:F2_OFF + (m + 1) * DM] = \
                W_ff2[m * 128:(m + 1) * 128, :]

        fblob = np.zeros((128, FF), np.float32)
        for b2 in range(2):
            fblob[:, A_OFF + b2 * NS:A_OFF + (b2 + 1) * NS] = \
                A[half * DI + b2 * 128:half * DI + (b2 + 1) * 128, :]
            fblob[:, BD_OFF + b2] = b_dt[half * DI + b2 * 128:
                                         half * DI + (b2 + 1) * 128]
            fblob[:, DP_OFF + b2] = Dp[half * DI + b2 * 128:
                                       half * DI + (b2 + 1) * 128]
        for dbi, db in enumerate(dbo):
            fblob[:, CB_OFF + dbi] = conv_b[db * 128:(db + 1) * 128]
        fblob[:, BF1_OFF:BF1_OFF + 8] = b_ff1.reshape(8, 128).T
        fblob[:, BF2_OFF:BF2_OFF + DM] = b_ff2[None, :]
        fblob[:, G2_OFF:G2_OFF + DM] = g2[None, :]
        fblob[:, BE2_OFF:BE2_OFF + DM] = beta2[None, :]
        fblob[:, SEL_OFF] = 1.0 - bwd
        fblob[:, SEL_OFF + 1] = float(bwd)

        in_maps.append(dict(
            xT=xpad.astype(BF),
            blob_bf=blob.astype(BF),
            blob_ffn=fb.astype(BF),
            blob_f32=fblob,
        ))
    return in_maps


def kernel(**inputs) -> np.ndarray:
    if "nc" not in _NC_CACHE:
        _NC_CACHE["nc"] = build_program()
    nc = _NC_CACHE["nc"]
    in_maps = _prep_inputs(inputs)
    res = run_bass_kernel_spmd(nc, in_maps, core_ids=list(range(8)))
    out = np.empty((B, L, DM), np.float32)
    for c in range(8):
        bb = c // 4
        r = c % 4
        out[bb, r * T_SLAB:(r + 1) * T_SLAB, :] = res.results[c]["out_slab"]
    return out
